# revision 10
# baseline (speedup 1.0000x reference)
"""Multi-head attention Trainium2 kernel (Bass/Tile), 8-core data-parallel.

Problem: B=8, N=2048, E=768, H=8 heads, D=96.
  q = x@Wq+bq; k = x@Wk+bk; v = x@Wv+bv  (per batch)
  energy = q @ k^T per head; att = softmax(energy)/sqrt(E); out = (att@v)@Wo + bo

Sharding: data-parallel over batch — each of the 8 cores handles one batch
element with a full copy of the weights. No collectives.

Per-core algorithm (all matmuls bf16 with fp32 PSUM accumulation):
  - x^T [E, N] is DMA'd in (host pre-transposes + casts bf16).
  - Q^T_h = Wq_h^T @ x^T + bq_h  per head  [96, 2048]   (bias = per-partition DVE add)
  - K^T_h = Wk_h^T @ x^T                   [96, 2048]   (bk dropped: softmax shift-invariant)
  - V' [N, 8*97]: per head block = [ones column | 96 data cols (x@Wv)].
  - Per head, per 1024-wide q window pair:
      energy^T[k_chunk, q] = (K^T_h chunk)^T @ Q^T_h    -> PSUM [128, 2x512]
      att = exp(energy^T)  (one ACT instr per [128,1024]; no max subtraction needed)
      out'^T [97, 512] += V'_h[k_chunk]^T @ att          (row 0 = softmax denominator)
      rb = 1/out'^T (full-partition reciprocal; row 0 = recip denominator)
      rbb = partition_broadcast(rb[0])  (GpSimd)
      outnorm^T_h = out'^T[1:97] * rbb   (bf16)
  - Next head's Q/K projection matmuls are interleaved into the attention loop
    so the in-order PE fills its slack while ACT (exp) is the local bottleneck.
  - Final: out[n_chunk, :] = sum_h outnorm^T_h[:, n_chunk]^T @ (Wo_h/sqrt(E))
  - Host adds bo_eff = bo + bv @ Wo / sqrt(E)  (exact because softmax rows sum to 1).
"""

import math
import sys
import types

import numpy as np
import ml_dtypes

B, N, E, H = 8, 2048, 768, 8
D = E // H          # 96
DP = D + 1          # 97: per-head V width incl. leading ones column
N_CORES = 8
NT = N // 128       # 16 row chunks of x / V
ET = E // 128       # 6 embedding chunks
QF = 512            # moving free-dim tile
NQF = N // QF       # 4 q windows
NQP = NQF // 2      # 2 q window pairs

_BF16 = ml_dtypes.bfloat16

_compiled = {}


def _install_ntff_hook_stub():
    """bass_utils imports antenv.axon_hooks when tracing; provide the glue if
    the image's antenv stub lacks it (harmless when trace=False)."""
    if "antenv.axon_hooks" in sys.modules:
        return
    hook = None
    try:
        from trn_agent_boot.trn_boot import _ntff_profile_via_ctypes

        hook = _ntff_profile_via_ctypes("/opt/axon/libaxon_pjrt.so")
    except Exception:
        pass
    mod = types.ModuleType("antenv.axon_hooks")
    mod.get_axon_ntff_profile_hook = lambda: hook
    mod.set_axon_ntff_profile_hook = lambda h: None
    sys.modules["antenv.axon_hooks"] = mod


def _build():
    import concourse.tile as tile
    import concourse.bacc as bacc
    from concourse import mybir

    bf = mybir.dt.bfloat16
    f32 = mybir.dt.float32
    Exp = mybir.ActivationFunctionType.Exp

    nc = bacc.Bacc("TRN2", target_bir_lowering=False, debug=False,
                   num_devices=N_CORES)

    xT_d = nc.dram_tensor("xT", [E, N], bf, kind="ExternalInput")
    wq_d = nc.dram_tensor("wq", [E, E], bf, kind="ExternalInput")
    wk_d = nc.dram_tensor("wk", [E, E], bf, kind="ExternalInput")
    wv_d = nc.dram_tensor("wv", [E, E], bf, kind="ExternalInput")
    wo_d = nc.dram_tensor("wo", [H * DP, E], bf, kind="ExternalInput")  # padded+scaled
    bq_d = nc.dram_tensor("bq", [E, 1], f32, kind="ExternalInput")
    out_d = nc.dram_tensor("out", [N, E], f32, kind="ExternalOutput")

    with tile.TileContext(nc) as tc:
        from contextlib import ExitStack

        with ExitStack() as ctx:
            const = ctx.enter_context(tc.tile_pool(name="const", bufs=1))
            vpool = ctx.enter_context(tc.tile_pool(name="vstore", bufs=1))
            qkpool = ctx.enter_context(tc.tile_pool(name="qk", bufs=2))
            onpool = ctx.enter_context(tc.tile_pool(name="onorm", bufs=1))
            att_pool = ctx.enter_context(tc.tile_pool(name="att", bufs=3))
            small = ctx.enter_context(tc.tile_pool(name="small", bufs=4))
            outsb_pool = ctx.enter_context(tc.tile_pool(name="outsb", bufs=3))

            # ---- persistent SBUF loads ----
            # Loads are ordered by first use and spread across the two HW DGE
            # queues (sync, scalar) + the GpSimd SW queue so startup is not
            # serialized on a single ~90GB/s queue. x^T is split into 512-col
            # window tiles so projections can start after window 0 lands.
            ldq = [nc.sync, nc.scalar, nc.gpsimd]
            qi = [0]

            def ld(dst_ap, src_ap):
                ldq[qi[0] % len(ldq)].dma_start(dst_ap, src_ap)
                qi[0] += 1

            bq_sb = []
            for h in range(H):
                t = const.tile([D, 1], f32, tag=f"bq{h}", name=f"bq{h}")
                nc.sync.dma_start(t[:], bq_d.ap()[h * D:(h + 1) * D, :])
                bq_sb.append(t)

            def load_w(dram, name):
                tiles = []
                for i in range(ET):
                    t = const.tile([128, E], bf, tag=f"{name}{i}", name=f"{name}{i}")
                    ld(t[:], dram.ap()[i * 128:(i + 1) * 128, :])
                    tiles.append(t)
                return tiles

            xTw = [[const.tile([128, QF], bf, tag=f"xT{i}_{w}", name=f"xT{i}_{w}")
                    for w in range(NQF)] for i in range(ET)]

            def load_xT_window(w):
                for i in range(ET):
                    ld(xTw[i][w][:],
                       xT_d.ap()[i * 128:(i + 1) * 128, w * QF:(w + 1) * QF])


            # Pairwise wq[i]/xTw[i][0] so the first projection matmuls (which
            # consume ein-tiles in order) can start as soon as each pair lands.
            wq = [const.tile([128, E], bf, tag=f"wq{i}", name=f"wq{i}")
                  for i in range(ET)]
            for i in range(ET):
                ld(wq[i][:], wq_d.ap()[i * 128:(i + 1) * 128, :])
                ld(xTw[i][0][:], xT_d.ap()[i * 128:(i + 1) * 128, 0:QF])
            wk = load_w(wk_d, "wk")
            for w in range(1, NQF):
                load_xT_window(w)
            wv = load_w(wv_d, "wv")

            wo = []
            for h in range(H):
                t = const.tile([DP, E], bf, tag=f"wo{h}", name=f"wo{h}")
                ld(t[:], wo_d.ap()[h * DP:(h + 1) * DP, :])
                wo.append(t)

            # ---- Phases 1+2 ----
            onorm = [onpool.tile([DP, N], bf, tag=f"on{h}", name=f"on{h}")
                     for h in range(H)]
            vtiles = []
            qkpsum_cm = tc.tile_pool(name="qkpsum", bufs=2, space="PSUM")
            with qkpsum_cm as qkpsum:

                def proj_tasks(h, qt, kt):
                    """Micro-tasks for head h's Q^T/K^T projections: one matmul
                    (or finishing DVE op) per yield. Window-interleaved so the
                    first q/k windows complete first."""
                    for qf in range(NQF):
                        for dst, w, bias in ((qt, wq, bq_sb[h]), (kt, wk, None)):
                            pq = qkpsum.tile([D, QF], f32, tag="pqk",
                                             name=f"pqk{h}_{qf}_{0 if bias is not None else 1}")
                            for ein in range(ET):
                                nc.tensor.matmul(
                                    pq[:],
                                    w[ein][:, h * D:(h + 1) * D],
                                    xTw[ein][qf][:],
                                    start=(ein == 0), stop=(ein == ET - 1),
                                )
                                yield
                            sl = dst[:, qf * QF:(qf + 1) * QF]
                            if bias is not None:
                                nc.vector.tensor_scalar_add(sl, pq[:], bias[:])
                            else:
                                nc.vector.tensor_copy(sl, pq[:])
                            yield

                def attention(h, qt, kt, next_tasks, epsum, opsum):
                    """Head h attention; drains next_tasks (next head's
                    projections) between inner iterations to fill PE slack."""
                    def drain(k):
                        for _ in range(k):
                            if next_tasks is None:
                                return
                            if next(next_tasks, "done") == "done":
                                return

                    for qp in range(NQP):
                        po = [opsum.tile([DP, QF], f32, tag="po",
                                         name=f"po{h}_{qp}_{j}")
                              for j in range(2)]
                        for kc in range(NT):
                            pe = epsum.tile([128, 2 * QF], f32, tag="pe",
                                            name=f"pe{h}_{qp}_{kc}")
                            for j in range(2):
                                nc.tensor.matmul(
                                    pe[:, j * QF:(j + 1) * QF],
                                    kt[:, kc * 128:(kc + 1) * 128],
                                    qt[:, (2 * qp + j) * QF:(2 * qp + j + 1) * QF],
                                    start=True, stop=True,
                                )
                            att = att_pool.tile([128, 2 * QF], bf, tag="att",
                                                name=f"att{h}_{qp}_{kc}")
                            nc.scalar.activation(att[:], pe[:], Exp)
                            for j in range(2):
                                nc.tensor.matmul(
                                    po[j][:],
                                    vtiles[kc][:, h * DP:(h + 1) * DP],
                                    att[:, j * QF:(j + 1) * QF],
                                    start=(kc == 0), stop=(kc == NT - 1),
                                )
                            drain(2)
                        for j in range(2):
                            qf = 2 * qp + j
                            rb = small.tile([DP, QF], f32, tag="rb",
                                            name=f"rb{h}_{qf}")
                            nc.vector.reciprocal_approx_fast(rb[:], po[j][:])
                            rbb = small.tile([DP, QF], f32, tag="rbb",
                                             name=f"rbb{h}_{qf}")
                            nc.gpsimd.partition_broadcast(rbb[:], rb[0:1, :])
                            nc.vector.tensor_mul(
                                onorm[h][:, qf * QF:(qf + 1) * QF],
                                po[j][:], rbb[:])
                            drain(1)

                # head 0 projections up front (DVE copies finish during the
                # V-projection phase); head h+1 interleaved with head h.
                qts, kts = {}, {}
                qts[0] = qkpool.tile([D, N], bf, tag="qt", name="qt0")
                kts[0] = qkpool.tile([D, N], bf, tag="kt", name="kt0")
                for _ in proj_tasks(0, qts[0], kts[0]):
                    pass

                # V' = [1 | x @ Wv] per head, [128, 8*97] per n-chunk
                with tc.tile_pool(name="vpsum", bufs=2, space="PSUM") as vpsum:
                    for nch in range(NT):
                        pv = vpsum.tile([128, E], f32, tag="pv", name=f"pv{nch}")
                        for f0, f1 in ((0, 512), (512, 768)):
                            for ein in range(ET):
                                nc.tensor.matmul(
                                    pv[:, f0:f1],
                                    xTw[ein][nch // 4][:, (nch % 4) * 128:
                                                       (nch % 4 + 1) * 128],
                                    wv[ein][:, f0:f1],
                                    start=(ein == 0), stop=(ein == ET - 1),
                                )
                        vt = vpool.tile([128, H * DP], bf, tag=f"v{nch}",
                                        name=f"v{nch}")
                        vview = vt[:].rearrange("p (h c) -> p h c", c=DP)
                        nc.vector.memset(vview[:, :, 0:1], 1.0)
                        nc.vector.tensor_copy(
                            vview[:, :, 1:DP],
                            pv[:].rearrange("p (h c) -> p h c", c=D),
                        )
                        vtiles.append(vt)

                with tc.tile_pool(name="epsum", bufs=2, space="PSUM") as epsum, \
                     tc.tile_pool(name="opsum", bufs=2, space="PSUM") as opsum:
                    for h in range(H):
                        tasks = None
                        if h + 1 < H:
                            qts[h + 1] = qkpool.tile([D, N], bf, tag="qt",
                                                     name=f"qt{h+1}")
                            kts[h + 1] = qkpool.tile([D, N], bf, tag="kt",
                                                     name=f"kt{h+1}")
                            tasks = proj_tasks(h + 1, qts[h + 1], kts[h + 1])
                        attention(h, qts[h], kts[h], tasks, epsum, opsum)
                        if tasks is not None:
                            for _ in tasks:  # finish any leftovers
                                pass
                        qts.pop(h), kts.pop(h)

            # ---- Phase 3: output projection ----
            with tc.tile_pool(name="fpsum", bufs=2, space="PSUM") as fpsum:
                for nch in range(NT):
                    pf = fpsum.tile([128, E], f32, tag="pf", name=f"pf{nch}")
                    for f0, f1 in ((0, 512), (512, 768)):
                        for h in range(H):
                            nc.tensor.matmul(
                                pf[:, f0:f1],
                                onorm[h][:, nch * 128:(nch + 1) * 128],
                                wo[h][:, f0:f1],
                                start=(h == 0), stop=(h == H - 1),
                            )
                    osb = outsb_pool.tile([128, E], f32, tag="osb",
                                          name=f"osb{nch}")
                    nc.vector.tensor_copy(osb[:], pf[:])
                    (nc.sync if nch % 2 == 0 else nc.scalar).dma_start(
                        out_d.ap()[nch * 128:(nch + 1) * 128, :], osb[:])

    nc.compile()
    return nc


def _get_nc():
    if "nc" not in _compiled:
        _install_ntff_hook_stub()
        _compiled["nc"] = _build()
    return _compiled["nc"]


def prepare_in_maps(x, Wq, Wk, Wv, Wo, bq):
    """Host-side prep: transpose/cast per-core inputs."""
    scale = np.float32(1.0 / math.sqrt(E))
    wq_b = np.ascontiguousarray(Wq.astype(_BF16))
    wk_b = np.ascontiguousarray(Wk.astype(_BF16))
    wv_b = np.ascontiguousarray(Wv.astype(_BF16))
    wo_pad = np.zeros((H * DP, E), np.float32)
    for h in range(H):
        wo_pad[h * DP + 1:(h + 1) * DP] = Wo[h * D:(h + 1) * D] * scale
    wo_b = np.ascontiguousarray(wo_pad.astype(_BF16))
    bq_c = np.ascontiguousarray(bq.astype(np.float32).reshape(E, 1))
    in_maps = []
    for c in range(N_CORES):
        in_maps.append({
            "xT": np.ascontiguousarray(x[c].T.astype(_BF16)),
            "wq": wq_b, "wk": wk_b, "wv": wv_b, "wo": wo_b,
            "bq": bq_c,
        })
    return in_maps


def run(x, Wq, bq, Wk, bk, Wv, bv, Wo, bo, trace=False, **spmd_kwargs):
    """Run on hardware; returns (out [B,N,E] fp32, BassKernelResults)."""
    from concourse.bass_utils import run_bass_kernel_spmd

    nc = _get_nc()
    in_maps = prepare_in_maps(x, Wq, Wk, Wv, Wo, bq)
    res = run_bass_kernel_spmd(nc, in_maps, core_ids=list(range(N_CORES)),
                               trace=trace, **spmd_kwargs)
    scale = np.float32(1.0 / math.sqrt(E))
    bo_eff = (bo.astype(np.float32)
              + (bv.astype(np.float32) @ Wo.astype(np.float32)) * scale)
    out = np.stack([res.results[c]["out"] for c in range(N_CORES)], axis=0)
    out = out + bo_eff[None, None, :]
    return out.astype(np.float32), res


def kernel(x, Wq, bq, Wk, bk, Wv, bv, Wo, bo):
    x = np.asarray(x); Wq = np.asarray(Wq); bq = np.asarray(bq)
    Wk = np.asarray(Wk); bk = np.asarray(bk); Wv = np.asarray(Wv)
    bv = np.asarray(bv); Wo = np.asarray(Wo); bo = np.asarray(bo)
    out, _ = run(x, Wq, bq, Wk, bk, Wv, bv, Wo, bo, trace=False)
    return out


# revision 11
# speedup vs baseline: 1.0137x; 1.0137x over previous
"""Multi-head attention Trainium2 kernel (Bass/Tile), 8-core data-parallel.

Problem: B=8, N=2048, E=768, H=8 heads, D=96.
  q = x@Wq+bq; k = x@Wk+bk; v = x@Wv+bv  (per batch)
  energy = q @ k^T per head; att = softmax(energy)/sqrt(E); out = (att@v)@Wo + bo

Sharding: data-parallel over batch — each of the 8 cores handles one batch
element with a full copy of the weights. No collectives.

Per-core algorithm (all matmuls bf16 with fp32 PSUM accumulation):
  - x^T [E, N] is DMA'd in (host pre-transposes + casts bf16).
  - Q^T_h = Wq_h^T @ x^T + bq_h  per head  [96, 2048]   (bias = per-partition DVE add)
  - K^T_h = Wk_h^T @ x^T                   [96, 2048]   (bk dropped: softmax shift-invariant)
  - V' [N, 8*97]: per head block = [ones column | 96 data cols (x@Wv)].
  - Per head, per 1024-wide q window pair:
      energy^T[k_chunk, q] = (K^T_h chunk)^T @ Q^T_h    -> PSUM [128, 2x512]
      att = exp(energy^T)  (one ACT instr per [128,1024]; no max subtraction needed)
      out'^T [97, 512] += V'_h[k_chunk]^T @ att          (row 0 = softmax denominator)
      rb = 1/out'^T (full-partition reciprocal; row 0 = recip denominator)
      rbb = partition_broadcast(rb[0])  (GpSimd)
      outnorm^T_h = out'^T[1:97] * rbb   (bf16)
  - Next head's Q/K projection matmuls are interleaved into the attention loop
    so the in-order PE fills its slack while ACT (exp) is the local bottleneck.
  - Final: out[n_chunk, :] = sum_h outnorm^T_h[:, n_chunk]^T @ (Wo_h/sqrt(E))
  - Host adds bo_eff = bo + bv @ Wo / sqrt(E)  (exact because softmax rows sum to 1).
"""

import math
import sys
import types

import numpy as np
import ml_dtypes

B, N, E, H = 8, 2048, 768, 8
D = E // H          # 96
DP = D + 1          # 97: per-head V width incl. leading ones column
N_CORES = 8
NT = N // 128       # 16 row chunks of x / V
ET = E // 128       # 6 embedding chunks
QF = 512            # moving free-dim tile
NQF = N // QF       # 4 q windows
NQP = NQF // 2      # 2 q window pairs

_BF16 = ml_dtypes.bfloat16

_compiled = {}


def _install_ntff_hook_stub():
    """bass_utils imports antenv.axon_hooks when tracing; provide the glue if
    the image's antenv stub lacks it (harmless when trace=False)."""
    if "antenv.axon_hooks" in sys.modules:
        return
    hook = None
    try:
        from trn_agent_boot.trn_boot import _ntff_profile_via_ctypes

        hook = _ntff_profile_via_ctypes("/opt/axon/libaxon_pjrt.so")
    except Exception:
        pass
    mod = types.ModuleType("antenv.axon_hooks")
    mod.get_axon_ntff_profile_hook = lambda: hook
    mod.set_axon_ntff_profile_hook = lambda h: None
    sys.modules["antenv.axon_hooks"] = mod


def _build():
    import concourse.tile as tile
    import concourse.bacc as bacc
    from concourse import mybir

    bf = mybir.dt.bfloat16
    f32 = mybir.dt.float32
    Exp = mybir.ActivationFunctionType.Exp

    nc = bacc.Bacc("TRN2", target_bir_lowering=False, debug=False,
                   num_devices=N_CORES)

    xT_d = nc.dram_tensor("xT", [E, N], bf, kind="ExternalInput")
    wq_d = nc.dram_tensor("wq", [E, E], bf, kind="ExternalInput")
    wk_d = nc.dram_tensor("wk", [E, E], bf, kind="ExternalInput")
    wv_d = nc.dram_tensor("wv", [E, E], bf, kind="ExternalInput")
    wo_d = nc.dram_tensor("wo", [H * DP, E], bf, kind="ExternalInput")  # padded+scaled
    bq_d = nc.dram_tensor("bq", [E, 1], f32, kind="ExternalInput")
    out_d = nc.dram_tensor("out", [N, E], f32, kind="ExternalOutput")

    with tile.TileContext(nc) as tc:
        from contextlib import ExitStack

        with ExitStack() as ctx:
            const = ctx.enter_context(tc.tile_pool(name="const", bufs=1))
            vpool = ctx.enter_context(tc.tile_pool(name="vstore", bufs=1))
            qkpool = ctx.enter_context(tc.tile_pool(name="qk", bufs=2))
            onpool = ctx.enter_context(tc.tile_pool(name="onorm", bufs=1))
            att_pool = ctx.enter_context(tc.tile_pool(name="att", bufs=3))
            small = ctx.enter_context(tc.tile_pool(name="small", bufs=4))
            outsb_pool = ctx.enter_context(tc.tile_pool(name="outsb", bufs=3))

            # ---- persistent SBUF loads ----
            # Loads are ordered by first use and spread across the two HW DGE
            # queues (sync, scalar) + the GpSimd SW queue so startup is not
            # serialized on a single ~90GB/s queue. x^T is split into 512-col
            # window tiles so projections can start after window 0 lands.
            ldq = [nc.sync, nc.scalar, nc.gpsimd]
            qi = [0]

            def ld(dst_ap, src_ap):
                ldq[qi[0] % len(ldq)].dma_start(dst_ap, src_ap)
                qi[0] += 1

            bq_sb = []
            for h in range(H):
                t = const.tile([D, 1], f32, tag=f"bq{h}", name=f"bq{h}")
                nc.sync.dma_start(t[:], bq_d.ap()[h * D:(h + 1) * D, :])
                bq_sb.append(t)

            def load_w(dram, name):
                tiles = []
                for i in range(ET):
                    t = const.tile([128, E], bf, tag=f"{name}{i}", name=f"{name}{i}")
                    ld(t[:], dram.ap()[i * 128:(i + 1) * 128, :])
                    tiles.append(t)
                return tiles

            xTw = [[const.tile([128, QF], bf, tag=f"xT{i}_{w}", name=f"xT{i}_{w}")
                    for w in range(NQF)] for i in range(ET)]

            def load_xT_window(w):
                for i in range(ET):
                    ld(xTw[i][w][:],
                       xT_d.ap()[i * 128:(i + 1) * 128, w * QF:(w + 1) * QF])


            # Pairwise wq[i]/xTw[i][0] so the first projection matmuls (which
            # consume ein-tiles in order) can start as soon as each pair lands.
            wq = [const.tile([128, E], bf, tag=f"wq{i}", name=f"wq{i}")
                  for i in range(ET)]
            for i in range(ET):
                ld(wq[i][:], wq_d.ap()[i * 128:(i + 1) * 128, :])
                ld(xTw[i][0][:], xT_d.ap()[i * 128:(i + 1) * 128, 0:QF])
            wk = load_w(wk_d, "wk")
            for w in range(1, NQF):
                load_xT_window(w)
            wv = load_w(wv_d, "wv")

            wo = []
            for h in range(H):
                t = const.tile([DP, E], bf, tag=f"wo{h}", name=f"wo{h}")
                ld(t[:], wo_d.ap()[h * DP:(h + 1) * DP, :])
                wo.append(t)

            # ---- Phases 1+2 ----
            onorm = [onpool.tile([DP, N], bf, tag=f"on{h}", name=f"on{h}")
                     for h in range(H)]
            vtiles = []
            qkpsum_cm = tc.tile_pool(name="qkpsum", bufs=2, space="PSUM")
            with qkpsum_cm as qkpsum:

                def proj_tasks(h, qt, kt):
                    """Micro-tasks for head h's Q^T/K^T projections: one matmul
                    (or finishing DVE op) per yield. Window-interleaved so the
                    first q/k windows complete first."""
                    for qf in range(NQF):
                        for dst, w, bias in ((qt, wq, bq_sb[h]), (kt, wk, None)):
                            pq = qkpsum.tile([D, QF], f32, tag="pqk",
                                             name=f"pqk{h}_{qf}_{0 if bias is not None else 1}")
                            for ein in range(ET):
                                nc.tensor.matmul(
                                    pq[:],
                                    w[ein][:, h * D:(h + 1) * D],
                                    xTw[ein][qf][:],
                                    start=(ein == 0), stop=(ein == ET - 1),
                                )
                                yield
                            sl = dst[:, qf * QF:(qf + 1) * QF]
                            if bias is not None:
                                nc.vector.tensor_scalar_add(sl, pq[:], bias[:])
                            else:
                                nc.vector.tensor_copy(sl, pq[:])
                            yield

                def attention(h, qt, kt, next_tasks, epsum, opsum,
                              defer_fill_first_pair=False):
                    """Head h attention; drains next_tasks (next head's
                    projections, or the tail of the output projection) between
                    inner iterations to fill PE slack."""
                    def drain(k, qp=1):
                        if defer_fill_first_pair and qp == 0:
                            return
                        for _ in range(k):
                            if next_tasks is None:
                                return
                            if next(next_tasks, "done") == "done":
                                return

                    for qp in range(NQP):
                        po = [opsum.tile([DP, QF], f32, tag="po",
                                         name=f"po{h}_{qp}_{j}")
                              for j in range(2)]
                        for kc in range(NT):
                            pe = epsum.tile([128, 2 * QF], f32, tag="pe",
                                            name=f"pe{h}_{qp}_{kc}")
                            for j in range(2):
                                nc.tensor.matmul(
                                    pe[:, j * QF:(j + 1) * QF],
                                    kt[:, kc * 128:(kc + 1) * 128],
                                    qt[:, (2 * qp + j) * QF:(2 * qp + j + 1) * QF],
                                    start=True, stop=True,
                                )
                            att = att_pool.tile([128, 2 * QF], bf, tag="att",
                                                name=f"att{h}_{qp}_{kc}")
                            nc.scalar.activation(att[:], pe[:], Exp)
                            for j in range(2):
                                nc.tensor.matmul(
                                    po[j][:],
                                    vtiles[kc][:, h * DP:(h + 1) * DP],
                                    att[:, j * QF:(j + 1) * QF],
                                    start=(kc == 0), stop=(kc == NT - 1),
                                )
                            drain(2, qp)
                        for j in range(2):
                            qf = 2 * qp + j
                            rb = small.tile([DP, QF], f32, tag="rb",
                                            name=f"rb{h}_{qf}")
                            nc.vector.reciprocal_approx_fast(rb[:], po[j][:])
                            rbb = small.tile([DP, QF], f32, tag="rbb",
                                             name=f"rbb{h}_{qf}")
                            nc.gpsimd.partition_broadcast(rbb[:], rb[0:1, :])
                            nc.vector.tensor_mul(
                                onorm[h][:, qf * QF:(qf + 1) * QF],
                                po[j][:], rbb[:])
                            drain(1, qp)

                # head 0 projections up front (DVE copies finish during the
                # V-projection phase); head h+1 interleaved with head h.
                qts, kts = {}, {}
                qts[0] = qkpool.tile([D, N], bf, tag="qt", name="qt0")
                kts[0] = qkpool.tile([D, N], bf, tag="kt", name="kt0")
                for _ in proj_tasks(0, qts[0], kts[0]):
                    pass

                # V' = [1 | x @ Wv] per head, [128, 8*97] per n-chunk
                with tc.tile_pool(name="vpsum", bufs=2, space="PSUM") as vpsum:
                    for nch in range(NT):
                        pv = vpsum.tile([128, E], f32, tag="pv", name=f"pv{nch}")
                        for f0, f1 in ((0, 512), (512, 768)):
                            for ein in range(ET):
                                nc.tensor.matmul(
                                    pv[:, f0:f1],
                                    xTw[ein][nch // 4][:, (nch % 4) * 128:
                                                       (nch % 4 + 1) * 128],
                                    wv[ein][:, f0:f1],
                                    start=(ein == 0), stop=(ein == ET - 1),
                                )
                        vt = vpool.tile([128, H * DP], bf, tag=f"v{nch}",
                                        name=f"v{nch}")
                        vview = vt[:].rearrange("p (h c) -> p h c", c=DP)
                        nc.vector.memset(vview[:, :, 0:1], 1.0)
                        nc.vector.tensor_copy(
                            vview[:, :, 1:DP],
                            pv[:].rearrange("p (h c) -> p h c", c=D),
                        )
                        vtiles.append(vt)

                def final_tasks(nchs):
                    """Output-projection micro-tasks: one matmul (or the
                    finishing copy/store) per yield. PSUM comes from the qkpsum
                    pool's 1-bank slots (idle once projections are done)."""
                    for nch in nchs:
                        osb = outsb_pool.tile([128, E], f32, tag="osb",
                                              name=f"osb{nch}")
                        for f0, f1 in ((0, 512), (512, 768)):
                            pf = qkpsum.tile([128, f1 - f0], f32, tag="pqk",
                                             name=f"pf{nch}_{f0}")
                            for h in range(H):
                                nc.tensor.matmul(
                                    pf[:],
                                    onorm[h][:, nch * 128:(nch + 1) * 128],
                                    wo[h][:, f0:f1],
                                    start=(h == 0), stop=(h == H - 1),
                                )
                                yield
                            nc.vector.tensor_copy(osb[:, f0:f1], pf[:])
                            yield
                        (nc.sync if nch % 2 == 0 else nc.scalar).dma_start(
                            out_d.ap()[nch * 128:(nch + 1) * 128, :], osb[:])

                final_rest = None
                with tc.tile_pool(name="epsum", bufs=2, space="PSUM") as epsum, \
                     tc.tile_pool(name="opsum", bufs=2, space="PSUM") as opsum:
                    for h in range(H):
                        if h + 1 < H:
                            qts[h + 1] = qkpool.tile([D, N], bf, tag="qt",
                                                     name=f"qt{h+1}")
                            kts[h + 1] = qkpool.tile([D, N], bf, tag="kt",
                                                     name=f"kt{h+1}")
                            tasks = proj_tasks(h + 1, qts[h + 1], kts[h + 1])
                        else:
                            # last head: fill PE slack with the first half of
                            # the output projection (n-chunks 0..7 only need
                            # head-7 windows 0/1, normalized in window pair 0).
                            tasks = final_tasks(range(8))
                        attention(h, qts[h], kts[h], tasks, epsum, opsum,
                                  defer_fill_first_pair=(h + 1 == H))
                        if tasks is not None:
                            for _ in tasks:  # finish any leftovers
                                pass
                        qts.pop(h), kts.pop(h)
                    final_rest = final_tasks(range(8, NT))
                    for _ in final_rest:
                        pass


    nc.compile()
    return nc


def _get_nc():
    if "nc" not in _compiled:
        _install_ntff_hook_stub()
        _compiled["nc"] = _build()
    return _compiled["nc"]


def prepare_in_maps(x, Wq, Wk, Wv, Wo, bq):
    """Host-side prep: transpose/cast per-core inputs."""
    scale = np.float32(1.0 / math.sqrt(E))
    wq_b = np.ascontiguousarray(Wq.astype(_BF16))
    wk_b = np.ascontiguousarray(Wk.astype(_BF16))
    wv_b = np.ascontiguousarray(Wv.astype(_BF16))
    wo_pad = np.zeros((H * DP, E), np.float32)
    for h in range(H):
        wo_pad[h * DP + 1:(h + 1) * DP] = Wo[h * D:(h + 1) * D] * scale
    wo_b = np.ascontiguousarray(wo_pad.astype(_BF16))
    bq_c = np.ascontiguousarray(bq.astype(np.float32).reshape(E, 1))
    in_maps = []
    for c in range(N_CORES):
        in_maps.append({
            "xT": np.ascontiguousarray(x[c].T.astype(_BF16)),
            "wq": wq_b, "wk": wk_b, "wv": wv_b, "wo": wo_b,
            "bq": bq_c,
        })
    return in_maps


def run(x, Wq, bq, Wk, bk, Wv, bv, Wo, bo, trace=False, **spmd_kwargs):
    """Run on hardware; returns (out [B,N,E] fp32, BassKernelResults)."""
    from concourse.bass_utils import run_bass_kernel_spmd

    nc = _get_nc()
    in_maps = prepare_in_maps(x, Wq, Wk, Wv, Wo, bq)
    res = run_bass_kernel_spmd(nc, in_maps, core_ids=list(range(N_CORES)),
                               trace=trace, **spmd_kwargs)
    scale = np.float32(1.0 / math.sqrt(E))
    bo_eff = (bo.astype(np.float32)
              + (bv.astype(np.float32) @ Wo.astype(np.float32)) * scale)
    out = np.stack([res.results[c]["out"] for c in range(N_CORES)], axis=0)
    out = out + bo_eff[None, None, :]
    return out.astype(np.float32), res


def kernel(x, Wq, bq, Wk, bk, Wv, bv, Wo, bo):
    x = np.asarray(x); Wq = np.asarray(Wq); bq = np.asarray(bq)
    Wk = np.asarray(Wk); bk = np.asarray(bk); Wv = np.asarray(Wv)
    bv = np.asarray(bv); Wo = np.asarray(Wo); bo = np.asarray(bo)
    out, _ = run(x, Wq, bq, Wk, bk, Wv, bv, Wo, bo, trace=False)
    return out


# revision 12
# speedup vs baseline: 1.0146x; 1.0009x over previous
"""Multi-head attention Trainium2 kernel (Bass/Tile), 8-core data-parallel.

Problem: B=8, N=2048, E=768, H=8 heads, D=96.
  q = x@Wq+bq; k = x@Wk+bk; v = x@Wv+bv  (per batch)
  energy = q @ k^T per head; att = softmax(energy)/sqrt(E); out = (att@v)@Wo + bo

Sharding: data-parallel over batch — each of the 8 cores handles one batch
element with a full copy of the weights. No collectives.

Per-core algorithm (all matmuls bf16 with fp32 PSUM accumulation):
  - x^T [E, N] is DMA'd in (host pre-transposes + casts bf16).
  - Q^T_h = Wq_h^T @ x^T + bq_h  per head  [96, 2048]   (bias = per-partition DVE add)
  - K^T_h = Wk_h^T @ x^T                   [96, 2048]   (bk dropped: softmax shift-invariant)
  - V' [N, 8*97]: per head block = [ones column | 96 data cols (x@Wv)].
  - Per head, per 1024-wide q window pair:
      energy^T[k_chunk, q] = (K^T_h chunk)^T @ Q^T_h    -> PSUM [128, 2x512]
      att = exp(energy^T)  (one ACT instr per [128,1024]; no max subtraction
        needed: |energy| < ~20 so fp32/bf16 exp cannot overflow)
      out'^T [97, 512] += V'_h[k_chunk]^T @ att          (row 0 = softmax denominator)
      rb = 1/out'^T  (reciprocal_approx_fast over all 97 partitions; only row 0
        -- the denominator -- is consumed)
      rbb[0:97] = partition_broadcast(rb[0])  (GpSimd, HW broadcasts partition 0)
      outnorm^T_h[0:97] = out'^T * rbb   (bf16; row 0 becomes 1.0, a dummy row)
  - The dummy row flows into the output projection against a host-padded Wo
    with a zero row per head block, so partition bases stay 32-aligned.
  - Next head's Q/K projection matmuls are interleaved into the attention loop
    so the in-order PE fills its slack while ACT (exp) is the local bottleneck;
    the last head's slack is filled with the first half of the output projection.
  - Final: out[n_chunk, :] = sum_h outnorm^T_h[:, n_chunk]^T @ (Wo_h/sqrt(E)),
    PSUM slots borrowed from the idle projection pool.
  - Host adds bo_eff = bo + bv @ Wo / sqrt(E)  (exact because softmax rows sum to 1).
"""

import math
import sys
import types

import numpy as np
import ml_dtypes

B, N, E, H = 8, 2048, 768, 8
D = E // H          # 96
DP = D + 1          # 97: per-head V width incl. leading ones column
N_CORES = 8
NT = N // 128       # 16 row chunks of x / V
ET = E // 128       # 6 embedding chunks
QF = 512            # moving free-dim tile
NQF = N // QF       # 4 q windows
NQP = NQF // 2      # 2 q window pairs

_BF16 = ml_dtypes.bfloat16

_compiled = {}


def _install_ntff_hook_stub():
    """bass_utils imports antenv.axon_hooks when tracing; provide the glue if
    the image's antenv stub lacks it (harmless when trace=False)."""
    if "antenv.axon_hooks" in sys.modules:
        return
    hook = None
    try:
        from trn_agent_boot.trn_boot import _ntff_profile_via_ctypes

        hook = _ntff_profile_via_ctypes("/opt/axon/libaxon_pjrt.so")
    except Exception:
        pass
    mod = types.ModuleType("antenv.axon_hooks")
    mod.get_axon_ntff_profile_hook = lambda: hook
    mod.set_axon_ntff_profile_hook = lambda h: None
    sys.modules["antenv.axon_hooks"] = mod


def _build():
    import concourse.tile as tile
    import concourse.bacc as bacc
    from concourse import mybir

    bf = mybir.dt.bfloat16
    f32 = mybir.dt.float32
    Exp = mybir.ActivationFunctionType.Exp

    nc = bacc.Bacc("TRN2", target_bir_lowering=False, debug=False,
                   num_devices=N_CORES)

    xT_d = nc.dram_tensor("xT", [E, N], bf, kind="ExternalInput")
    wq_d = nc.dram_tensor("wq", [E, E], bf, kind="ExternalInput")
    wk_d = nc.dram_tensor("wk", [E, E], bf, kind="ExternalInput")
    wv_d = nc.dram_tensor("wv", [E, E], bf, kind="ExternalInput")
    wo_d = nc.dram_tensor("wo", [H * DP, E], bf, kind="ExternalInput")  # padded+scaled
    bq_d = nc.dram_tensor("bq", [E, 1], f32, kind="ExternalInput")
    out_d = nc.dram_tensor("out", [N, E], f32, kind="ExternalOutput")

    with tile.TileContext(nc) as tc:
        from contextlib import ExitStack

        with ExitStack() as ctx:
            const = ctx.enter_context(tc.tile_pool(name="const", bufs=1))
            vpool = ctx.enter_context(tc.tile_pool(name="vstore", bufs=1))
            qkpool = ctx.enter_context(tc.tile_pool(name="qk", bufs=2))
            onpool = ctx.enter_context(tc.tile_pool(name="onorm", bufs=1))
            att_pool = ctx.enter_context(tc.tile_pool(name="att", bufs=3))
            small = ctx.enter_context(tc.tile_pool(name="small", bufs=4))
            outsb_pool = ctx.enter_context(tc.tile_pool(name="outsb", bufs=3))

            # ---- persistent SBUF loads ----
            # Loads are ordered by first use and spread across the two HW DGE
            # queues (sync, scalar) + the GpSimd SW queue so startup is not
            # serialized on a single ~90GB/s queue. x^T is split into 512-col
            # window tiles so projections can start after window 0 lands.
            ldq = [nc.sync, nc.scalar, nc.gpsimd]
            qi = [0]

            def ld(dst_ap, src_ap):
                ldq[qi[0] % len(ldq)].dma_start(dst_ap, src_ap)
                qi[0] += 1

            bq_sb = []
            for h in range(H):
                t = const.tile([D, 1], f32, tag=f"bq{h}", name=f"bq{h}")
                nc.sync.dma_start(t[:], bq_d.ap()[h * D:(h + 1) * D, :])
                bq_sb.append(t)

            def load_w(dram, name):
                tiles = []
                for i in range(ET):
                    t = const.tile([128, E], bf, tag=f"{name}{i}", name=f"{name}{i}")
                    ld(t[:], dram.ap()[i * 128:(i + 1) * 128, :])
                    tiles.append(t)
                return tiles

            xTw = [[const.tile([128, QF], bf, tag=f"xT{i}_{w}", name=f"xT{i}_{w}")
                    for w in range(NQF)] for i in range(ET)]

            def load_xT_window(w):
                for i in range(ET):
                    ld(xTw[i][w][:],
                       xT_d.ap()[i * 128:(i + 1) * 128, w * QF:(w + 1) * QF])


            # Pairwise wq[i]/xTw[i][0] so the first projection matmuls (which
            # consume ein-tiles in order) can start as soon as each pair lands.
            wq = [const.tile([128, E], bf, tag=f"wq{i}", name=f"wq{i}")
                  for i in range(ET)]
            for i in range(ET):
                ld(wq[i][:], wq_d.ap()[i * 128:(i + 1) * 128, :])
                ld(xTw[i][0][:], xT_d.ap()[i * 128:(i + 1) * 128, 0:QF])
            wk = load_w(wk_d, "wk")
            for w in range(1, NQF):
                load_xT_window(w)
            wv = load_w(wv_d, "wv")

            wo = []
            for h in range(H):
                t = const.tile([DP, E], bf, tag=f"wo{h}", name=f"wo{h}")
                ld(t[:], wo_d.ap()[h * DP:(h + 1) * DP, :])
                wo.append(t)

            # ---- Phases 1+2 ----
            onorm = [onpool.tile([DP, N], bf, tag=f"on{h}", name=f"on{h}")
                     for h in range(H)]
            vtiles = []
            qkpsum_cm = tc.tile_pool(name="qkpsum", bufs=2, space="PSUM")
            with qkpsum_cm as qkpsum:

                def proj_tasks(h, qt, kt):
                    """Micro-tasks for head h's Q^T/K^T projections: one matmul
                    (or finishing DVE op) per yield. Window-interleaved so the
                    first q/k windows complete first."""
                    for qf in range(NQF):
                        for dst, w, bias in ((qt, wq, bq_sb[h]), (kt, wk, None)):
                            pq = qkpsum.tile([D, QF], f32, tag="pqk",
                                             name=f"pqk{h}_{qf}_{0 if bias is not None else 1}")
                            for ein in range(ET):
                                nc.tensor.matmul(
                                    pq[:],
                                    w[ein][:, h * D:(h + 1) * D],
                                    xTw[ein][qf][:],
                                    start=(ein == 0), stop=(ein == ET - 1),
                                )
                                yield
                            sl = dst[:, qf * QF:(qf + 1) * QF]
                            if bias is not None:
                                nc.vector.tensor_scalar_add(sl, pq[:], bias[:])
                            else:
                                nc.vector.tensor_copy(sl, pq[:])
                            yield

                def attention(h, qt, kt, next_tasks, epsum, opsum,
                              defer_fill_first_pair=False):
                    """Head h attention; drains next_tasks (next head's
                    projections, or the tail of the output projection) between
                    inner iterations to fill PE slack."""
                    def drain(k, qp=1):
                        if defer_fill_first_pair and qp == 0:
                            return
                        for _ in range(k):
                            if next_tasks is None:
                                return
                            if next(next_tasks, "done") == "done":
                                return

                    for qp in range(NQP):
                        po = [opsum.tile([DP, QF], f32, tag="po",
                                         name=f"po{h}_{qp}_{j}")
                              for j in range(2)]
                        for kc in range(NT):
                            pe = epsum.tile([128, 2 * QF], f32, tag="pe",
                                            name=f"pe{h}_{qp}_{kc}")
                            for j in range(2):
                                nc.tensor.matmul(
                                    pe[:, j * QF:(j + 1) * QF],
                                    kt[:, kc * 128:(kc + 1) * 128],
                                    qt[:, (2 * qp + j) * QF:(2 * qp + j + 1) * QF],
                                    start=True, stop=True,
                                )
                            att = att_pool.tile([128, 2 * QF], bf, tag="att",
                                                name=f"att{h}_{qp}_{kc}")
                            nc.scalar.activation(att[:], pe[:], Exp)
                            for j in range(2):
                                nc.tensor.matmul(
                                    po[j][:],
                                    vtiles[kc][:, h * DP:(h + 1) * DP],
                                    att[:, j * QF:(j + 1) * QF],
                                    start=(kc == 0), stop=(kc == NT - 1),
                                )
                            drain(2, qp)
                        for j in range(2):
                            qf = 2 * qp + j
                            rb = small.tile([DP, QF], f32, tag="rb",
                                            name=f"rb{h}_{qf}")
                            nc.vector.reciprocal_approx_fast(rb[:], po[j][:])
                            rbb = small.tile([DP, QF], f32, tag="rbb",
                                             name=f"rbb{h}_{qf}")
                            nc.gpsimd.partition_broadcast(rbb[:], rb[0:1, :])
                            nc.vector.tensor_mul(
                                onorm[h][:, qf * QF:(qf + 1) * QF],
                                po[j][:], rbb[:])
                            drain(1, qp)

                # head 0 projections up front (DVE copies finish during the
                # V-projection phase); head h+1 interleaved with head h.
                qts, kts = {}, {}
                qts[0] = qkpool.tile([D, N], bf, tag="qt", name="qt0")
                kts[0] = qkpool.tile([D, N], bf, tag="kt", name="kt0")
                for _ in proj_tasks(0, qts[0], kts[0]):
                    pass

                # V' = [1 | x @ Wv] per head, [128, 8*97] per n-chunk
                with tc.tile_pool(name="vpsum", bufs=2, space="PSUM") as vpsum:
                    for nch in range(NT):
                        pv = vpsum.tile([128, E], f32, tag="pv", name=f"pv{nch}")
                        for f0, f1 in ((0, 512), (512, 768)):
                            for ein in range(ET):
                                nc.tensor.matmul(
                                    pv[:, f0:f1],
                                    xTw[ein][nch // 4][:, (nch % 4) * 128:
                                                       (nch % 4 + 1) * 128],
                                    wv[ein][:, f0:f1],
                                    start=(ein == 0), stop=(ein == ET - 1),
                                )
                        vt = vpool.tile([128, H * DP], bf, tag=f"v{nch}",
                                        name=f"v{nch}")
                        vview = vt[:].rearrange("p (h c) -> p h c", c=DP)
                        nc.vector.memset(vview[:, :, 0:1], 1.0)
                        nc.vector.tensor_copy(
                            vview[:, :, 1:DP],
                            pv[:].rearrange("p (h c) -> p h c", c=D),
                        )
                        vtiles.append(vt)

                def final_tasks(nchs):
                    """Output-projection micro-tasks: one matmul (or the
                    finishing copy/store) per yield. PSUM comes from the qkpsum
                    pool's 1-bank slots (idle once projections are done)."""
                    for nch in nchs:
                        osb = outsb_pool.tile([128, E], f32, tag="osb",
                                              name=f"osb{nch}")
                        for f0, f1 in ((0, 512), (512, 768)):
                            pf = qkpsum.tile([128, f1 - f0], f32, tag="pqk",
                                             name=f"pf{nch}_{f0}")
                            for h in range(H):
                                nc.tensor.matmul(
                                    pf[:],
                                    onorm[h][:, nch * 128:(nch + 1) * 128],
                                    wo[h][:, f0:f1],
                                    start=(h == 0), stop=(h == H - 1),
                                )
                                yield
                            nc.vector.tensor_copy(osb[:, f0:f1], pf[:])
                            yield
                        (nc.sync if nch % 2 == 0 else nc.scalar).dma_start(
                            out_d.ap()[nch * 128:(nch + 1) * 128, :], osb[:])

                final_rest = None
                with tc.tile_pool(name="epsum", bufs=2, space="PSUM") as epsum, \
                     tc.tile_pool(name="opsum", bufs=2, space="PSUM") as opsum:
                    for h in range(H):
                        if h + 1 < H:
                            qts[h + 1] = qkpool.tile([D, N], bf, tag="qt",
                                                     name=f"qt{h+1}")
                            kts[h + 1] = qkpool.tile([D, N], bf, tag="kt",
                                                     name=f"kt{h+1}")
                            tasks = proj_tasks(h + 1, qts[h + 1], kts[h + 1])
                        else:
                            # last head: fill PE slack with the first half of
                            # the output projection (n-chunks 0..7 only need
                            # head-7 windows 0/1, normalized in window pair 0).
                            tasks = final_tasks(range(8))
                        attention(h, qts[h], kts[h], tasks, epsum, opsum,
                                  defer_fill_first_pair=(h + 1 == H))
                        if tasks is not None:
                            for _ in tasks:  # finish any leftovers
                                pass
                        qts.pop(h), kts.pop(h)
                    final_rest = final_tasks(range(8, NT))
                    for _ in final_rest:
                        pass


    nc.compile()
    return nc


def _get_nc():
    if "nc" not in _compiled:
        _install_ntff_hook_stub()
        _compiled["nc"] = _build()
    return _compiled["nc"]


def prepare_in_maps(x, Wq, Wk, Wv, Wo, bq):
    """Host-side prep: transpose/cast per-core inputs."""
    scale = np.float32(1.0 / math.sqrt(E))
    wq_b = np.ascontiguousarray(Wq.astype(_BF16))
    wk_b = np.ascontiguousarray(Wk.astype(_BF16))
    wv_b = np.ascontiguousarray(Wv.astype(_BF16))
    wo_pad = np.zeros((H * DP, E), np.float32)
    for h in range(H):
        wo_pad[h * DP + 1:(h + 1) * DP] = Wo[h * D:(h + 1) * D] * scale
    wo_b = np.ascontiguousarray(wo_pad.astype(_BF16))
    bq_c = np.ascontiguousarray(bq.astype(np.float32).reshape(E, 1))
    in_maps = []
    for c in range(N_CORES):
        in_maps.append({
            "xT": np.ascontiguousarray(x[c].T.astype(_BF16)),
            "wq": wq_b, "wk": wk_b, "wv": wv_b, "wo": wo_b,
            "bq": bq_c,
        })
    return in_maps


def run(x, Wq, bq, Wk, bk, Wv, bv, Wo, bo, trace=False, **spmd_kwargs):
    """Run on hardware; returns (out [B,N,E] fp32, BassKernelResults)."""
    from concourse.bass_utils import run_bass_kernel_spmd

    nc = _get_nc()
    in_maps = prepare_in_maps(x, Wq, Wk, Wv, Wo, bq)
    res = run_bass_kernel_spmd(nc, in_maps, core_ids=list(range(N_CORES)),
                               trace=trace, **spmd_kwargs)
    scale = np.float32(1.0 / math.sqrt(E))
    bo_eff = (bo.astype(np.float32)
              + (bv.astype(np.float32) @ Wo.astype(np.float32)) * scale)
    out = np.stack([res.results[c]["out"] for c in range(N_CORES)], axis=0)
    out = out + bo_eff[None, None, :]
    return out.astype(np.float32), res


def kernel(x, Wq, bq, Wk, bk, Wv, bv, Wo, bo):
    x = np.asarray(x); Wq = np.asarray(Wq); bq = np.asarray(bq)
    Wk = np.asarray(Wk); bk = np.asarray(bk); Wv = np.asarray(Wv)
    bv = np.asarray(bv); Wo = np.asarray(Wo); bo = np.asarray(bo)
    out, _ = run(x, Wq, bq, Wk, bk, Wv, bv, Wo, bo, trace=False)
    return out
